# revision 19
# baseline (speedup 1.0000x reference)
"""Trainium2 Bass kernel for AdaptivePooling + NMS (nn_AdaptivePoolingAndNMS).

Reference semantics:
  x = input_tensor.sum(axis=1)                               # [B, 28, 28]
  scores_k = avgpool2d(x, k, stride 1, VALID).reshape(B, -1) # k in {4, 6, 8}
  all_scores = concat(scores_k)                              # [B, 1595]
  per group g: greedy NMS picking N_list[g] windows (IoU thresh)
  returns (proposal_indices [B,6] i32, proposal_scores [B,6] f32,
           window_scores [B,1595] f32)

Sharding: pure data parallel, batch dim over 8 cores (2 per core).

Device algorithm per core (B_loc = 2): see inline comments. Grid rows of the
three pooled maps are stacked on partitions at 32-aligned offsets (0/32/64)
because engine access patterns may only start at partitions 0/32/64/96.
"""

import functools

import numpy as np

H = W = 28
B, C = 16, 2048
N_CORES = 8
B_LOC = B // N_CORES
HW = H * W
CHUNKS = C // 128          # 16 K-chunks of 128 channels
CHUNKS_PER_DMA = 2         # 2 chunks per DMA tile -> [128, 2*784] = 802 KB
N_DMAS = CHUNKS // CHUNKS_PER_DMA
RPAD = 96                  # padded row count for transposed NMS tiles
BIGE = 65536.0             # encoding offset (exact in f32 for small ints)
PEN = 1.0e9                # suppression penalty / padding


def _expected_grid_coords(wns, ns, ks, stride=16):
    coords = []
    for g in range(len(ns)):
        n, k = ns[g], ks[g]
        ii, jj = np.meshgrid(np.arange(n), np.arange(n), indexing="ij")
        boxes = (
            np.stack([jj, ii, jj + k, ii + k], -1).reshape(-1, 4).astype(np.float32)
            * stride
        )
        coords.append(boxes)
    return np.concatenate(coords, 0)


def _host_consts(cfg):
    """Build the small constant arrays uploaded to each core."""
    wns, nl, ths = cfg
    n_groups = len(nl)
    ns, ks = [], []
    for g in range(n_groups):
        size = wns[g + 1] - wns[g]
        n = int(round(np.sqrt(size)))
        assert n * n == size, f"group {g} size {size} not square"
        assert n <= 32
        ns.append(n)
        ks.append(H - n + 1)
    goff = [32 * g for g in range(n_groups)]   # partition-aligned group rows
    RP = goff[-1] + ns[-1]                     # total padded rows (85)
    assert RP <= 128

    # vertical pooling matrix A_cat [28, RP] (zero cols on gap rows)
    acat = np.zeros((H, RP), np.float32)
    for g in range(n_groups):
        n, k = ns[g], ks[g]
        for i in range(n):
            acat[i : i + k, goff[g] + i] = 1.0

    ident = np.eye(128, dtype=np.float32)

    nmax = max(ns)
    # cgrid [RP, 4*nmax + 6]: packed per-row constants
    #   0:nmax        CIOTA_MB = j - BIGE  (batch 0)
    #   nmax:2nmax    CIOTA_MB (batch 1)
    #   2n:3n         CIOTA    = j         (batch 0)
    #   3n:4n         CIOTA    (batch 1)
    #   4n+0          ROWBASE64 = 64*row_in_group
    #   4n+1          IOTAROW   = row_in_group
    #   4n+2          K         = k_g
    #   4n+3          TAU       = 2*t*k^2/(1+t)
    #   4n+4          N_AP      = n_g
    #   4n+5          WNS_AP    = wns_g
    cg = np.zeros((RP, 4 * nmax + 6), np.float32)
    for r in range(RP):
        cg[r, 0:nmax] = np.arange(nmax) - BIGE
        cg[r, nmax : 2 * nmax] = np.arange(nmax) - BIGE
        cg[r, 2 * nmax : 3 * nmax] = np.arange(nmax)
        cg[r, 3 * nmax : 4 * nmax] = np.arange(nmax)
    for g in range(n_groups):
        n, k, t = ns[g], ks[g], ths[g]
        tau = 2.0 * t * k * k / (1.0 + t)
        for i in range(n):
            r = goff[g] + i
            cg[r, 4 * nmax + 0] = 64.0 * i
            cg[r, 4 * nmax + 1] = float(i)
            cg[r, 4 * nmax + 2] = float(k)
            cg[r, 4 * nmax + 3] = tau
            cg[r, 4 * nmax + 4] = float(n)
            cg[r, 4 * nmax + 5] = float(wns[g])

    # inde/indr [2*n_groups, RP]: broadcast selectors for enc rows (0..G-1)
    # and r rows (G..2G-1) of the transposed F tile
    inde = np.zeros((2 * n_groups, RP), np.float32)
    indr = np.zeros((2 * n_groups, RP), np.float32)
    for g in range(n_groups):
        inde[g, goff[g] : goff[g] + ns[g]] = 1.0
        indr[n_groups + g, goff[g] : goff[g] + ns[g]] = 1.0

    # crow [2, RPAD]: value at free pos p = row_in_group(p) - BIGE
    crow = np.full((2, RPAD), 0.0, np.float32)
    for g in range(n_groups):
        crow[:, goff[g] : goff[g] + ns[g]] = np.arange(ns[g]) - BIGE

    return {
        "ns": ns,
        "ks": ks,
        "goff": goff,
        "RP": RP,
        "nmax": nmax,
        "acat": acat,
        "ident": ident,
        "cgrid": cg,
        "inde": inde,
        "indr": indr,
        "crow": crow,
    }


@functools.lru_cache(maxsize=4)
def _build(cfg):
    """Build + compile the per-core Bass program. cfg = (wns, nl, ths)."""
    import concourse.bacc as bacc
    import concourse.mybir as mybir
    from concourse.tile import TileContext

    wns, nl, ths = cfg
    hc = _host_consts(cfg)
    ns, ks, goff = hc["ns"], hc["ks"], hc["goff"]
    RP, nmax = hc["RP"], hc["nmax"]
    n_groups = len(nl)
    n_iter = max(nl)
    n_prop = sum(nl)
    slot_base = [sum(nl[:g]) for g in range(n_groups)]
    wtot = wns[-1]

    f32 = mybir.dt.float32
    i32 = mybir.dt.int32

    nc = bacc.Bacc(None, target_bir_lowering=False, debug=False)

    x_in = nc.declare_dram_parameter("x", [B_LOC, C, HW], f32, isOutput=False)
    acat_in = nc.declare_dram_parameter("acat", [H, RP], f32, isOutput=False)
    ident_in = nc.declare_dram_parameter("ident", [128, 128], f32, isOutput=False)
    cgrid_in = nc.declare_dram_parameter(
        "cgrid", [RP, 4 * nmax + 6], f32, isOutput=False
    )
    inde_in = nc.declare_dram_parameter(
        "inde", [2 * n_groups, RP], f32, isOutput=False
    )
    indr_in = nc.declare_dram_parameter(
        "indr", [2 * n_groups, RP], f32, isOutput=False
    )
    crow_in = nc.declare_dram_parameter("crow", [2, RPAD], f32, isOutput=False)

    pi_out = nc.declare_dram_parameter("pi", [B_LOC, n_prop], i32, isOutput=True)
    ps_out = nc.declare_dram_parameter("ps", [B_LOC, n_prop], f32, isOutput=True)
    ws_out = nc.declare_dram_parameter("ws", [B_LOC, wtot], f32, isOutput=True)

    AX = mybir.AxisListType.X
    OP = mybir.AluOpType
    ACTF = mybir.ActivationFunctionType

    with TileContext(nc) as tc:
        with (
            tc.tile_pool(name="const", bufs=1) as cpool,
            tc.tile_pool(name="xin", bufs=16) as xpool,
            tc.tile_pool(name="pairp", bufs=4) as ppool,
            tc.tile_pool(name="work", bufs=1) as wpool,
            tc.tile_pool(name="nms", bufs=2) as npool,
            tc.tile_pool(name="psacc", bufs=2, space="PSUM") as pacc,
            tc.tile_pool(name="psvp", bufs=1, space="PSUM") as pvp,
            tc.tile_pool(name="psnms", bufs=1, space="PSUM") as pnms,
        ):
            # ---- constants ----
            ones = cpool.tile([128, 1], f32)
            nc.vector.memset(ones[:], 1.0)
            acat = cpool.tile([H, RP], f32)
            nc.sync.dma_start(acat[:], acat_in[:])
            ident = cpool.tile([128, 128], f32)
            nc.sync.dma_start(ident[:], ident_in[:])
            cgrid = cpool.tile([RP, 4 * nmax + 6], f32)
            nc.sync.dma_start(cgrid[:], cgrid_in[:])
            inde = cpool.tile([2 * n_groups, RP], f32)
            nc.sync.dma_start(inde[:], inde_in[:])
            indr = cpool.tile([2 * n_groups, RP], f32)
            nc.sync.dma_start(indr[:], indr_in[:])
            crow = cpool.tile([2, RPAD], f32)
            nc.sync.dma_start(crow[:], crow_in[:])

            ciota_mb = cgrid[:, 0 : 2 * nmax]
            ciota = cgrid[:, 2 * nmax : 4 * nmax]
            rowbase64 = cgrid[:, 4 * nmax : 4 * nmax + 1]
            iotarow = cgrid[:, 4 * nmax + 1 : 4 * nmax + 2]
            k_ap = cgrid[:, 4 * nmax + 2 : 4 * nmax + 3]
            tau_ap = cgrid[:, 4 * nmax + 3 : 4 * nmax + 4]
            n_ap = cgrid[:, 4 * nmax + 4 : 4 * nmax + 5]
            wns_ap = cgrid[:, 4 * nmax + 5 : 4 * nmax + 6]

            # ---- S: score grid [RP, 2, nmax] (free = (batch, col)) ----
            S = wpool.tile([RP, B_LOC, nmax], f32)
            nc.vector.memset(S[:], -PEN)

            # ---- channel sum + pooling, per batch ----
            for b in range(B_LOC):
                xb = x_in[b].rearrange("(a p) m -> p a m", p=128)  # [128,16,784]
                tiles = []
                for d in range(N_DMAS):
                    xt = xpool.tile([128, CHUNKS_PER_DMA, HW], f32, tag="xt")
                    # alternate the two HWDGE rings (sync + scalar queues)
                    eng = nc.sync if d % 2 == 0 else nc.scalar
                    eng.dma_start(
                        xt[:], xb[:, d * CHUNKS_PER_DMA : (d + 1) * CHUNKS_PER_DMA, :]
                    )
                    tiles.append(xt)

                # DVE add tree: 16 chunks -> 8 pairs -> 4 quads; then PE
                # accumulates the 4 quads (fp32 ones-matmul, 2-pass each)
                pairs = []
                for d in range(N_DMAS):
                    pair = ppool.tile([128, HW], f32, tag="pair")
                    nc.vector.tensor_add(
                        pair[:], tiles[d][:, 0, :], tiles[d][:, 1, :]
                    )
                    pairs.append(pair)
                acc0 = pacc.tile([1, HW // 2], f32, tag="acc")
                acc1 = pacc.tile([1, HW // 2], f32, tag="acc")
                nquad = N_DMAS // 2
                for qd in range(nquad):
                    quad = ppool.tile([128, HW], f32, tag="quad")
                    nc.vector.tensor_add(
                        quad[:], pairs[2 * qd][:], pairs[2 * qd + 1][:]
                    )
                    nc.tensor.matmul(
                        acc0[:],
                        ones[:],
                        quad[:, 0 : HW // 2],
                        start=(qd == 0),
                        stop=(qd == nquad - 1),
                    )
                    nc.tensor.matmul(
                        acc1[:],
                        ones[:],
                        quad[:, HW // 2 : HW],
                        start=(qd == 0),
                        stop=(qd == nquad - 1),
                    )
                # reshape [1, 784] -> [28, 28] (PSUM -> SBUF flat -> DMA reshape)
                xflat = wpool.tile([1, HW], f32, tag="xflat")
                nc.scalar.copy(xflat[:, 0 : HW // 2], acc0[:])
                nc.scalar.copy(xflat[:, HW // 2 : HW], acc1[:])
                xs = wpool.tile([H, W], f32, tag="xs")
                nc.sync.dma_start(xs[:], xflat[:])

                # vertical pooling (gap rows get zero columns in acat)
                vp = pvp.tile([RP, W], f32, tag="vp")
                nc.tensor.matmul(vp[:], acat[:], xs[:], start=True, stop=True)

                # horizontal pooling (shifted adds) into S[:, b, :]
                vps = wpool.tile([RP, W], f32, tag="vps")
                nc.vector.tensor_copy(vps[:], vp[:])
                t = wpool.tile([RP, W - 1], f32, tag="hp_t")
                nc.vector.tensor_add(t[:], vps[:, 0 : W - 1], vps[:, 1:W])
                s4 = wpool.tile([RP, W - 3], f32, tag="hp_s4")
                nc.vector.tensor_add(s4[:], t[:, 0 : W - 3], t[:, 2 : W - 1])
                for g in range(n_groups):
                    n, k = ns[g], ks[g]
                    sl = slice(goff[g], goff[g] + n)
                    if k == 4:
                        src = s4[sl, 0:n]
                    else:
                        tmp = wpool.tile([RP, nmax], f32, tag=f"hp_k{k}")
                        if k == 6:
                            nc.vector.tensor_add(
                                tmp[sl, 0:n], s4[sl, 0:n], t[sl, 4 : 4 + n]
                            )
                        elif k == 8:
                            nc.vector.tensor_add(
                                tmp[sl, 0:n], s4[sl, 0:n], s4[sl, 4 : 4 + n]
                            )
                        else:
                            raise NotImplementedError(f"k={k}")
                        src = tmp[sl, 0:n]
                    nc.scalar.mul(S[sl, b, 0:n], src, 1.0 / (k * k))

            # ---- window_scores out ----
            for g in range(n_groups):
                n = ns[g]
                dst = ws_out[:, wns[g] : wns[g] + n * n].rearrange(
                    "b (i j) -> i b j", i=n
                )
                nc.sync.dma_start(dst, S[goff[g] : goff[g] + n, :, 0:n])

            # ---- NMS ----
            # working copy so the ws DMAs don't gate suppression writes
            Sw = wpool.tile([RP, B_LOC, nmax], f32, tag="Sw")
            nc.vector.tensor_copy(Sw[:], S[:])
            S3 = Sw[:]  # [RP, 2, nmax]

            stage_sc = npool.tile([B_LOC, n_prop], f32, tag="stage_sc")
            stage85 = npool.tile([RP, 2 * n_iter], f32, tag="stage85")
            # persistent padded argmax tiles ([RPAD, 2]; pad rows preset once)
            rm = wpool.tile([RPAD, B_LOC], f32, tag="rm")
            nc.vector.memset(rm[:], -PEN)
            en = wpool.tile([RPAD, B_LOC], f32, tag="en")
            nc.vector.memset(en[:], 0.0)

            for it in range(n_iter):
                # per-row max + argmax-encoding enc' = 64r + c - BIGE
                nc.vector.tensor_reduce(rm[0:RP, :], S3, axis=AX, op=OP.max)
                eq = npool.tile([RP, B_LOC, nmax], f32, tag="eq")
                nc.vector.tensor_tensor(
                    eq[:],
                    S3,
                    rm[0:RP, :].unsqueeze(2).broadcast_to([RP, B_LOC, nmax]),
                    op=OP.is_equal,
                )
                tm = npool.tile([RP, B_LOC, nmax], f32, tag="tm")
                nc.vector.tensor_tensor(
                    tm[:], eq[:], ciota_mb.rearrange("r (b j) -> r b j", b=B_LOC),
                    op=OP.mult,
                )
                nc.vector.tensor_reduce(en[0:RP, :], tm[:], axis=AX, op=OP.min)
                nc.vector.tensor_scalar(
                    en[0:RP, :], en[0:RP, :], rowbase64, None, op0=OP.add
                )
                # transpose to [2, RPAD]; per-group reductions read PSUM direct
                Q1 = pnms.tile([B_LOC, RPAD], f32, tag="Q1")
                nc.tensor.transpose(Q1[:], rm[:], ident[0:RPAD, 0:RPAD])
                Q2 = pnms.tile([B_LOC, RPAD], f32, tag="Q2")
                nc.tensor.transpose(Q2[:], en[:], ident[0:RPAD, 0:RPAD])
                q1g = Q1[:].rearrange("b (g n) -> b g n", g=n_groups)
                q2g = Q2[:].rearrange("b (g n) -> b g n", g=n_groups)

                m2 = npool.tile([B_LOC, n_groups], f32, tag="m2")
                nc.vector.tensor_reduce(m2[:], q1g, axis=AX, op=OP.max)
                eq2 = npool.tile([B_LOC, n_groups, 32], f32, tag="eq2")
                nc.vector.tensor_tensor(
                    eq2[:],
                    q1g,
                    m2[:].unsqueeze(2).broadcast_to([B_LOC, n_groups, 32]),
                    op=OP.is_equal,
                )
                # F [2, 2G]: cols 0..G-1 = enc'-min, cols G..2G-1 = r'-min
                F = npool.tile([B_LOC, 2 * n_groups], f32, tag="F")
                t3 = npool.tile([B_LOC, n_groups, 32], f32, tag="t3")
                nc.vector.tensor_tensor(t3[:], eq2[:], q2g, op=OP.mult)
                nc.vector.tensor_reduce(
                    F[:, 0:n_groups], t3[:], axis=AX, op=OP.min
                )
                t4 = npool.tile([B_LOC, n_groups, 32], f32, tag="t4")
                nc.vector.tensor_tensor(
                    t4[:],
                    eq2[:],
                    crow[:].rearrange("b (g n) -> b g n", g=n_groups),
                    op=OP.mult,
                )
                nc.vector.tensor_reduce(
                    F[:, n_groups : 2 * n_groups], t4[:], axis=AX, op=OP.min
                )
                # stage proposal scores for still-active groups
                for g in range(n_groups):
                    if it < nl[g]:
                        nc.scalar.copy(
                            stage_sc[:, slot_base[g] + it : slot_base[g] + it + 1],
                            m2[:, g : g + 1],
                        )

                # transpose F -> [2G, 2]; +BIGE; broadcast enc and r to rows
                Ft = pnms.tile([2 * n_groups, 2], f32, tag="Ft")
                nc.tensor.transpose(Ft[:], F[:], ident[0:2, 0:2])
                fc = npool.tile([2 * n_groups, 2], f32, tag="fc")
                nc.vector.tensor_scalar(fc[:], Ft[:], BIGE, None, op0=OP.add)
                Renc = pnms.tile([RP, B_LOC], f32, tag="Renc")
                nc.tensor.matmul(Renc[:], inde[:], fc[:], start=True, stop=True)
                Rr = pnms.tile([RP, B_LOC], f32, tag="Rr")
                nc.tensor.matmul(Rr[:], indr[:], fc[:], start=True, stop=True)
                rrs = npool.tile([RP, B_LOC], f32, tag="rrs")
                nc.vector.tensor_copy(rrs[:], Rr[:])
                # c = enc - 64 r
                rcs = npool.tile([RP, B_LOC], f32, tag="rcs")
                nc.vector.scalar_tensor_tensor(
                    rcs[:], rrs[:], -64.0, Renc[:], op0=OP.mult, op1=OP.add
                )
                # output indices: oi = r*n_g + wns_g + c (valid on group rows)
                oi = npool.tile([RP, B_LOC], f32, tag="oi")
                nc.vector.tensor_scalar(
                    oi[:], rrs[:], n_ap, wns_ap, op0=OP.mult, op1=OP.add
                )
                nc.vector.tensor_tensor(
                    stage85[:, B_LOC * it : B_LOC * (it + 1)],
                    oi[:],
                    rcs[:],
                    op=OP.add,
                )

                if it == n_iter - 1:
                    break

                # suppression: (k-|i-r|)+ * (k-|j-c|)+ > tau  -> S -= PEN
                # (computed sign-flipped: min(|d|-k, 0) * min(|e|-k, 0))
                d_ = npool.tile([RP, B_LOC], f32, tag="d_")
                nc.vector.tensor_tensor(
                    d_[:], iotarow.broadcast_to([RP, B_LOC]), rrs[:], op=OP.subtract
                )
                ad = npool.tile([RP, B_LOC], f32, tag="ad")
                nc.vector.scalar_tensor_tensor(
                    ad[:], d_[:], -1.0, d_[:], op0=OP.mult, op1=OP.max
                )  # |i-r|
                u = npool.tile([RP, B_LOC], f32, tag="u")
                nc.vector.tensor_scalar(
                    u[:], ad[:], k_ap, 0.0, op0=OP.subtract, op1=OP.min
                )  # min(|i-r|-k, 0)
                ci3 = ciota.rearrange("r (b j) -> r b j", b=B_LOC)
                rc3 = rcs[:].unsqueeze(2).broadcast_to([RP, B_LOC, nmax])
                D = npool.tile([RP, B_LOC, nmax], f32, tag="D")
                nc.vector.tensor_tensor(D[:], ci3, rc3, op=OP.subtract)
                aD = npool.tile([RP, B_LOC, nmax], f32, tag="aD")
                nc.vector.scalar_tensor_tensor(
                    aD[:], D[:], -1.0, D[:], op0=OP.mult, op1=OP.max
                )  # |j-c|
                tv = npool.tile([RP, B_LOC, nmax], f32, tag="tv")
                nc.vector.tensor_scalar(
                    tv[:], aD[:], k_ap, 0.0, op0=OP.subtract, op1=OP.min
                )  # min(|j-c|-k, 0)
                sp = npool.tile([RP, B_LOC, nmax], f32, tag="sp")
                nc.vector.tensor_tensor(
                    sp[:], tv[:], u[:].unsqueeze(2).broadcast_to([RP, B_LOC, nmax]),
                    op=OP.mult,
                )
                pen = npool.tile([RP, B_LOC, nmax], f32, tag="pen")
                nc.vector.tensor_scalar(
                    pen[:], sp[:], tau_ap, -PEN, op0=OP.is_gt, op1=OP.mult
                )
                nc.vector.tensor_tensor(S3, S3, pen[:], op=OP.add)

            # ---- stage outputs ----
            nc.sync.dma_start(ps_out[:], stage_sc[:, 0:n_prop])
            stage_i = npool.tile([RP, 2 * n_iter], i32, tag="stage_i")
            nc.vector.tensor_copy(stage_i[:], stage85[:])
            for g in range(n_groups):
                if nl[g] == 0:
                    continue
                src = stage_i[goff[g] : goff[g] + 1, :].rearrange(
                    "p (t b) -> p t b", b=B_LOC
                )
                for bb in range(B_LOC):
                    nc.sync.dma_start(
                        pi_out[bb : bb + 1, slot_base[g] : slot_base[g] + nl[g]],
                        src[:, 0 : nl[g], bb],
                    )

    nc.compile()
    return nc, hc


def _prepare(cfg, input_tensor, consts):
    in_maps = []
    for core in range(N_CORES):
        shard = np.ascontiguousarray(
            input_tensor[core * B_LOC : (core + 1) * B_LOC].reshape(B_LOC, C, HW)
        ).astype(np.float32, copy=False)
        m = {
            "x": shard,
            "acat": consts["acat"],
            "ident": consts["ident"],
            "cgrid": consts["cgrid"],
            "inde": consts["inde"],
            "indr": consts["indr"],
            "crow": consts["crow"],
        }
        in_maps.append(m)
    return in_maps


def kernel(
    num_proposals,
    input_tensor,
    window_nums_sum,
    N_list,
    iou_thresholds,
    coordinates_cat,
):
    from concourse.bass_utils import run_bass_kernel_spmd

    input_tensor = np.asarray(input_tensor)
    wns = tuple(int(v) for v in np.asarray(window_nums_sum))
    nl = tuple(int(v) for v in np.asarray(N_list))
    ths = tuple(float(v) for v in np.asarray(iou_thresholds))
    cfg = (wns, nl, ths)

    # sanity: boxes must be the regular grid this kernel specializes for
    hc_probe = _host_consts(cfg)
    exp = _expected_grid_coords(wns, hc_probe["ns"], hc_probe["ks"])
    cc = np.asarray(coordinates_cat, dtype=np.float32)
    assert cc.shape == exp.shape and np.allclose(cc, exp), (
        "coordinates_cat is not the expected sliding-window grid"
    )

    nc, _hc = _build(cfg)
    in_maps = _prepare(cfg, input_tensor, _hc)
    res = run_bass_kernel_spmd(nc, in_maps, core_ids=list(range(N_CORES))).results

    n_prop = sum(nl)
    wtot = wns[-1]
    pi = np.concatenate([res[i]["pi"] for i in range(N_CORES)], 0).astype(np.int32)
    ps = np.concatenate([res[i]["ps"] for i in range(N_CORES)], 0).astype(np.float32)
    ws = np.concatenate([res[i]["ws"] for i in range(N_CORES)], 0).astype(np.float32)
    assert pi.shape == (B, n_prop) and ws.shape == (B, wtot)
    return pi, ps, ws


# revision 22
# speedup vs baseline: 1.0169x; 1.0169x over previous
"""Trainium2 Bass kernel for AdaptivePooling + NMS (nn_AdaptivePoolingAndNMS).

Reference semantics:
  x = input_tensor.sum(axis=1)                               # [B, 28, 28]
  scores_k = avgpool2d(x, k, stride 1, VALID).reshape(B, -1) # k in {4, 6, 8}
  all_scores = concat(scores_k)                              # [B, 1595]
  per group g: greedy NMS picking N_list[g] windows (IoU thresh)
  returns (proposal_indices [B,6] i32, proposal_scores [B,6] f32,
           window_scores [B,1595] f32)

Sharding: pure data parallel, batch dim over 8 cores (2 per core).

Device algorithm per core (B_loc = 2): see inline comments. Grid rows of the
three pooled maps are stacked on partitions at 32-aligned offsets (0/32/64)
because engine access patterns may only start at partitions 0/32/64/96.
"""

import functools

import numpy as np

H = W = 28
B, C = 16, 2048
N_CORES = 8
B_LOC = B // N_CORES
HW = H * W
CHUNKS = C // 128          # 16 K-chunks of 128 channels
CPT = 4                    # channels per partition per DMA tile (contiguous)
N_DMAS = C // (128 * CPT)  # 4 DMAs per batch, [128, 4, 784] = 1.57 MB each
RPAD = 96                  # padded row count for transposed NMS tiles
BIGE = 65536.0             # encoding offset (exact in f32 for small ints)
PEN = 1.0e9                # suppression penalty / padding


def _expected_grid_coords(wns, ns, ks, stride=16):
    coords = []
    for g in range(len(ns)):
        n, k = ns[g], ks[g]
        ii, jj = np.meshgrid(np.arange(n), np.arange(n), indexing="ij")
        boxes = (
            np.stack([jj, ii, jj + k, ii + k], -1).reshape(-1, 4).astype(np.float32)
            * stride
        )
        coords.append(boxes)
    return np.concatenate(coords, 0)


def _host_consts(cfg):
    """Build the small constant arrays uploaded to each core."""
    wns, nl, ths = cfg
    n_groups = len(nl)
    ns, ks = [], []
    for g in range(n_groups):
        size = wns[g + 1] - wns[g]
        n = int(round(np.sqrt(size)))
        assert n * n == size, f"group {g} size {size} not square"
        assert n <= 32
        ns.append(n)
        ks.append(H - n + 1)
    goff = [32 * g for g in range(n_groups)]   # partition-aligned group rows
    RP = goff[-1] + ns[-1]                     # total padded rows (85)
    assert RP <= 128

    # vertical pooling matrix A_cat [28, RP] (zero cols on gap rows)
    acat = np.zeros((H, RP), np.float32)
    for g in range(n_groups):
        n, k = ns[g], ks[g]
        for i in range(n):
            acat[i : i + k, goff[g] + i] = 1.0

    ident = np.eye(128, dtype=np.float32)

    nmax = max(ns)
    # cgrid [RP, 4*nmax + 6]: packed per-row constants
    #   0:nmax        CIOTA_MB = j - BIGE  (batch 0)
    #   nmax:2nmax    CIOTA_MB (batch 1)
    #   2n:3n         CIOTA    = j         (batch 0)
    #   3n:4n         CIOTA    (batch 1)
    #   4n+0          ROWBASE64 = 64*row_in_group
    #   4n+1          IOTAROW   = row_in_group
    #   4n+2          K         = k_g
    #   4n+3          TAU       = 2*t*k^2/(1+t)
    #   4n+4          N_AP      = n_g
    #   4n+5          WNS_AP    = wns_g
    cg = np.zeros((RP, 4 * nmax + 6), np.float32)
    for r in range(RP):
        cg[r, 0:nmax] = np.arange(nmax) - BIGE
        cg[r, nmax : 2 * nmax] = np.arange(nmax) - BIGE
        cg[r, 2 * nmax : 3 * nmax] = np.arange(nmax)
        cg[r, 3 * nmax : 4 * nmax] = np.arange(nmax)
    for g in range(n_groups):
        n, k, t = ns[g], ks[g], ths[g]
        tau = 2.0 * t * k * k / (1.0 + t)
        for i in range(n):
            r = goff[g] + i
            cg[r, 4 * nmax + 0] = 64.0 * i
            cg[r, 4 * nmax + 1] = float(i)
            cg[r, 4 * nmax + 2] = float(k)
            cg[r, 4 * nmax + 3] = tau
            cg[r, 4 * nmax + 4] = float(n)
            cg[r, 4 * nmax + 5] = float(wns[g])

    # inde/indr [2*n_groups, RP]: broadcast selectors for enc rows (0..G-1)
    # and r rows (G..2G-1) of the transposed F tile
    inde = np.zeros((2 * n_groups, RP), np.float32)
    indr = np.zeros((2 * n_groups, RP), np.float32)
    for g in range(n_groups):
        inde[g, goff[g] : goff[g] + ns[g]] = 1.0
        indr[n_groups + g, goff[g] : goff[g] + ns[g]] = 1.0

    # crow [2, RPAD]: value at free pos p = row_in_group(p) - BIGE
    crow = np.full((2, RPAD), 0.0, np.float32)
    for g in range(n_groups):
        crow[:, goff[g] : goff[g] + ns[g]] = np.arange(ns[g]) - BIGE

    return {
        "ns": ns,
        "ks": ks,
        "goff": goff,
        "RP": RP,
        "nmax": nmax,
        "acat": acat,
        "ident": ident,
        "cgrid": cg,
        "inde": inde,
        "indr": indr,
        "crow": crow,
    }


@functools.lru_cache(maxsize=4)
def _build(cfg):
    """Build + compile the per-core Bass program. cfg = (wns, nl, ths)."""
    import concourse.bacc as bacc
    import concourse.mybir as mybir
    from concourse.tile import TileContext

    wns, nl, ths = cfg
    hc = _host_consts(cfg)
    ns, ks, goff = hc["ns"], hc["ks"], hc["goff"]
    RP, nmax = hc["RP"], hc["nmax"]
    n_groups = len(nl)
    n_iter = max(nl)
    n_prop = sum(nl)
    slot_base = [sum(nl[:g]) for g in range(n_groups)]
    wtot = wns[-1]

    f32 = mybir.dt.float32
    i32 = mybir.dt.int32

    nc = bacc.Bacc(None, target_bir_lowering=False, debug=False)

    x_in = nc.declare_dram_parameter("x", [B_LOC, C, HW], f32, isOutput=False)
    acat_in = nc.declare_dram_parameter("acat", [H, RP], f32, isOutput=False)
    ident_in = nc.declare_dram_parameter("ident", [128, 128], f32, isOutput=False)
    cgrid_in = nc.declare_dram_parameter(
        "cgrid", [RP, 4 * nmax + 6], f32, isOutput=False
    )
    inde_in = nc.declare_dram_parameter(
        "inde", [2 * n_groups, RP], f32, isOutput=False
    )
    indr_in = nc.declare_dram_parameter(
        "indr", [2 * n_groups, RP], f32, isOutput=False
    )
    crow_in = nc.declare_dram_parameter("crow", [2, RPAD], f32, isOutput=False)

    pir_out = nc.declare_dram_parameter(
        "pir", [RP, 2 * n_iter], f32, isOutput=True
    )
    ps_out = nc.declare_dram_parameter("ps", [B_LOC, n_prop], f32, isOutput=True)
    ws_out = nc.declare_dram_parameter("ws", [B_LOC, wtot], f32, isOutput=True)

    AX = mybir.AxisListType.X
    OP = mybir.AluOpType
    ACTF = mybir.ActivationFunctionType

    with TileContext(nc) as tc:
        with (
            tc.tile_pool(name="const", bufs=1) as cpool,
            tc.tile_pool(name="xin", bufs=8) as xpool,
            tc.tile_pool(name="pairp", bufs=2) as ppool,
            tc.tile_pool(name="work", bufs=1) as wpool,
            tc.tile_pool(name="nms", bufs=2) as npool,
            tc.tile_pool(name="psacc", bufs=2, space="PSUM") as pacc,
            tc.tile_pool(name="psvp", bufs=1, space="PSUM") as pvp,
            tc.tile_pool(name="psnms", bufs=1, space="PSUM") as pnms,
        ):
            # ---- constants ----
            ones = cpool.tile([128, 1], f32)
            nc.vector.memset(ones[:], 1.0)
            acat = cpool.tile([H, RP], f32)
            nc.sync.dma_start(acat[:], acat_in[:])
            ident = cpool.tile([128, 128], f32)
            nc.sync.dma_start(ident[:], ident_in[:])
            cgrid = cpool.tile([RP, 4 * nmax + 6], f32)
            nc.sync.dma_start(cgrid[:], cgrid_in[:])
            inde = cpool.tile([2 * n_groups, RP], f32)
            nc.sync.dma_start(inde[:], inde_in[:])
            indr = cpool.tile([2 * n_groups, RP], f32)
            nc.sync.dma_start(indr[:], indr_in[:])
            crow = cpool.tile([2, RPAD], f32)
            nc.sync.dma_start(crow[:], crow_in[:])

            ciota_mb = cgrid[:, 0 : 2 * nmax]
            ciota = cgrid[:, 2 * nmax : 4 * nmax]
            rowbase64 = cgrid[:, 4 * nmax : 4 * nmax + 1]
            iotarow = cgrid[:, 4 * nmax + 1 : 4 * nmax + 2]
            k_ap = cgrid[:, 4 * nmax + 2 : 4 * nmax + 3]
            tau_ap = cgrid[:, 4 * nmax + 3 : 4 * nmax + 4]
            n_ap = cgrid[:, 4 * nmax + 4 : 4 * nmax + 5]
            wns_ap = cgrid[:, 4 * nmax + 5 : 4 * nmax + 6]

            # ---- S: score grid [RP, 2, nmax] (free = (batch, col)) ----
            S = wpool.tile([RP, B_LOC, nmax], f32)
            nc.vector.memset(S[:], -PEN)

            # ---- channel sum + pooling, per batch ----
            for b in range(B_LOC):
                # partition p of DMA tile j holds channels 512j+4p..512j+4p+3
                # (contiguous per partition -> 1 DMA descriptor per partition)
                xb = x_in[b].rearrange("(j p ci) m -> j p ci m", p=128, ci=CPT)
                tiles = []
                for d in range(N_DMAS):
                    xt = xpool.tile([128, CPT, HW], f32, tag="xt")
                    # alternate the two HWDGE rings (sync + scalar queues)
                    eng = nc.sync if d % 2 == 0 else nc.scalar
                    eng.dma_start(xt[:], xb[d])
                    tiles.append(xt)

                # DVE add tree per tile: 4 chans -> 2 pairs -> 1 quad; then PE
                # accumulates the 4 quads (fp32 ones-matmul, 2-pass each)
                acc0 = pacc.tile([1, HW // 2], f32, tag="acc")
                acc1 = pacc.tile([1, HW // 2], f32, tag="acc")
                for d in range(N_DMAS):
                    pa = ppool.tile([128, HW], f32, tag="pa")
                    nc.vector.tensor_add(pa[:], tiles[d][:, 0, :], tiles[d][:, 1, :])
                    pb = ppool.tile([128, HW], f32, tag="pb")
                    nc.vector.tensor_add(pb[:], tiles[d][:, 2, :], tiles[d][:, 3, :])
                    quad = ppool.tile([128, HW], f32, tag="quad")
                    nc.vector.tensor_add(quad[:], pa[:], pb[:])
                    nc.tensor.matmul(
                        acc0[:],
                        ones[:],
                        quad[:, 0 : HW // 2],
                        start=(d == 0),
                        stop=(d == N_DMAS - 1),
                    )
                    nc.tensor.matmul(
                        acc1[:],
                        ones[:],
                        quad[:, HW // 2 : HW],
                        start=(d == 0),
                        stop=(d == N_DMAS - 1),
                    )
                # reshape [1, 784] -> [28, 28] (PSUM -> SBUF flat -> DMA reshape)
                xflat = wpool.tile([1, HW], f32, tag="xflat")
                nc.vector.tensor_copy(xflat[:, 0 : HW // 2], acc0[:])
                nc.vector.tensor_copy(xflat[:, HW // 2 : HW], acc1[:])
                xs = wpool.tile([H, W], f32, tag="xs")
                nc.sync.dma_start(xs[:], xflat[:])

                # vertical pooling (gap rows get zero columns in acat)
                vp = pvp.tile([RP, W], f32, tag="vp")
                nc.tensor.matmul(vp[:], acat[:], xs[:], start=True, stop=True)

                # horizontal pooling (shifted adds) into S[:, b, :]
                vps = wpool.tile([RP, W], f32, tag="vps")
                nc.vector.tensor_copy(vps[:], vp[:])
                t = wpool.tile([RP, W - 1], f32, tag="hp_t")
                nc.vector.tensor_add(t[:], vps[:, 0 : W - 1], vps[:, 1:W])
                s4 = wpool.tile([RP, W - 3], f32, tag="hp_s4")
                nc.vector.tensor_add(s4[:], t[:, 0 : W - 3], t[:, 2 : W - 1])
                for g in range(n_groups):
                    n, k = ns[g], ks[g]
                    sl = slice(goff[g], goff[g] + n)
                    if k == 4:
                        src = s4[sl, 0:n]
                    else:
                        tmp = wpool.tile([RP, nmax], f32, tag=f"hp_k{k}")
                        if k == 6:
                            nc.vector.tensor_add(
                                tmp[sl, 0:n], s4[sl, 0:n], t[sl, 4 : 4 + n]
                            )
                        elif k == 8:
                            nc.vector.tensor_add(
                                tmp[sl, 0:n], s4[sl, 0:n], s4[sl, 4 : 4 + n]
                            )
                        else:
                            raise NotImplementedError(f"k={k}")
                        src = tmp[sl, 0:n]
                    nc.scalar.mul(S[sl, b, 0:n], src, 1.0 / (k * k))

            # ---- window_scores out ----
            for g in range(n_groups):
                n = ns[g]
                dst = ws_out[:, wns[g] : wns[g] + n * n].rearrange(
                    "b (i j) -> i b j", i=n
                )
                nc.sync.dma_start(dst, S[goff[g] : goff[g] + n, :, 0:n])

            # ---- NMS ----
            # working copy so the ws DMAs don't gate suppression writes
            Sw = wpool.tile([RP, B_LOC, nmax], f32, tag="Sw")
            nc.vector.tensor_copy(Sw[:], S[:])
            S3 = Sw[:]  # [RP, 2, nmax]

            stage_sc = npool.tile([B_LOC, n_prop], f32, tag="stage_sc")
            stage85 = npool.tile([RP, 2 * n_iter], f32, tag="stage85")
            # persistent padded argmax tiles ([RPAD, 2]; pad rows preset once)
            rm = wpool.tile([RPAD, B_LOC], f32, tag="rm")
            nc.vector.memset(rm[:], -PEN)
            en = wpool.tile([RPAD, B_LOC], f32, tag="en")
            nc.vector.memset(en[:], 0.0)

            for it in range(n_iter):
                # per-row max + argmax-encoding enc' = 64r + c - BIGE
                nc.vector.tensor_reduce(rm[0:RP, :], S3, axis=AX, op=OP.max)
                eq = npool.tile([RP, B_LOC, nmax], f32, tag="eq")
                nc.vector.tensor_tensor(
                    eq[:],
                    S3,
                    rm[0:RP, :].unsqueeze(2).broadcast_to([RP, B_LOC, nmax]),
                    op=OP.is_equal,
                )
                tm = npool.tile([RP, B_LOC, nmax], f32, tag="tm")
                nc.vector.tensor_tensor(
                    tm[:], eq[:], ciota_mb.rearrange("r (b j) -> r b j", b=B_LOC),
                    op=OP.mult,
                )
                nc.vector.tensor_reduce(en[0:RP, :], tm[:], axis=AX, op=OP.min)
                nc.vector.tensor_scalar(
                    en[0:RP, :], en[0:RP, :], rowbase64, None, op0=OP.add
                )
                # transpose to [2, RPAD]; per-group reductions read PSUM direct
                Q1 = pnms.tile([B_LOC, RPAD], f32, tag="Q1")
                nc.tensor.transpose(Q1[:], rm[:], ident[0:RPAD, 0:RPAD])
                Q2 = pnms.tile([B_LOC, RPAD], f32, tag="Q2")
                nc.tensor.transpose(Q2[:], en[:], ident[0:RPAD, 0:RPAD])
                q1g = Q1[:].rearrange("b (g n) -> b g n", g=n_groups)
                q2g = Q2[:].rearrange("b (g n) -> b g n", g=n_groups)

                m2 = npool.tile([B_LOC, n_groups], f32, tag="m2")
                nc.vector.tensor_reduce(m2[:], q1g, axis=AX, op=OP.max)
                eq2 = npool.tile([B_LOC, n_groups, 32], f32, tag="eq2")
                nc.vector.tensor_tensor(
                    eq2[:],
                    q1g,
                    m2[:].unsqueeze(2).broadcast_to([B_LOC, n_groups, 32]),
                    op=OP.is_equal,
                )
                # F [2, 2G]: cols 0..G-1 = enc'-min, cols G..2G-1 = r'-min
                F = npool.tile([B_LOC, 2 * n_groups], f32, tag="F")
                t3 = npool.tile([B_LOC, n_groups, 32], f32, tag="t3")
                nc.vector.tensor_tensor(t3[:], eq2[:], q2g, op=OP.mult)
                nc.vector.tensor_reduce(
                    F[:, 0:n_groups], t3[:], axis=AX, op=OP.min
                )
                t4 = npool.tile([B_LOC, n_groups, 32], f32, tag="t4")
                nc.vector.tensor_tensor(
                    t4[:],
                    eq2[:],
                    crow[:].rearrange("b (g n) -> b g n", g=n_groups),
                    op=OP.mult,
                )
                nc.vector.tensor_reduce(
                    F[:, n_groups : 2 * n_groups], t4[:], axis=AX, op=OP.min
                )
                # stage proposal scores for still-active groups
                for g in range(n_groups):
                    if it < nl[g]:
                        nc.scalar.copy(
                            stage_sc[:, slot_base[g] + it : slot_base[g] + it + 1],
                            m2[:, g : g + 1],
                        )

                # transpose F -> [2G, 2]; +BIGE; broadcast enc and r to rows
                Ft = pnms.tile([2 * n_groups, 2], f32, tag="Ft")
                nc.tensor.transpose(Ft[:], F[:], ident[0:2, 0:2])
                fc = npool.tile([2 * n_groups, 2], f32, tag="fc")
                nc.vector.tensor_scalar(fc[:], Ft[:], BIGE, None, op0=OP.add)
                Renc = pnms.tile([RP, B_LOC], f32, tag="Renc")
                nc.tensor.matmul(Renc[:], inde[:], fc[:], start=True, stop=True)
                Rr = pnms.tile([RP, B_LOC], f32, tag="Rr")
                nc.tensor.matmul(Rr[:], indr[:], fc[:], start=True, stop=True)
                rrs = npool.tile([RP, B_LOC], f32, tag="rrs")
                nc.vector.tensor_copy(rrs[:], Rr[:])
                # c = enc - 64 r
                rcs = npool.tile([RP, B_LOC], f32, tag="rcs")
                nc.vector.scalar_tensor_tensor(
                    rcs[:], rrs[:], -64.0, Renc[:], op0=OP.mult, op1=OP.add
                )
                # output indices: oi = r*n_g + wns_g + c (valid on group rows)
                oi = npool.tile([RP, B_LOC], f32, tag="oi")
                nc.vector.tensor_scalar(
                    oi[:], rrs[:], n_ap, wns_ap, op0=OP.mult, op1=OP.add
                )
                nc.vector.tensor_tensor(
                    stage85[:, B_LOC * it : B_LOC * (it + 1)],
                    oi[:],
                    rcs[:],
                    op=OP.add,
                )

                if it == n_iter - 1:
                    break

                # suppression: (k-|i-r|)+ * (k-|j-c|)+ > tau  -> S -= PEN
                # (computed sign-flipped: min(|d|-k, 0) * min(|e|-k, 0))
                d_ = npool.tile([RP, B_LOC], f32, tag="d_")
                nc.vector.tensor_tensor(
                    d_[:], iotarow.broadcast_to([RP, B_LOC]), rrs[:], op=OP.subtract
                )
                ad = npool.tile([RP, B_LOC], f32, tag="ad")
                nc.vector.scalar_tensor_tensor(
                    ad[:], d_[:], -1.0, d_[:], op0=OP.mult, op1=OP.max
                )  # |i-r|
                u = npool.tile([RP, B_LOC], f32, tag="u")
                nc.vector.tensor_scalar(
                    u[:], ad[:], k_ap, 0.0, op0=OP.subtract, op1=OP.min
                )  # min(|i-r|-k, 0)
                ci3 = ciota.rearrange("r (b j) -> r b j", b=B_LOC)
                rc3 = rcs[:].unsqueeze(2).broadcast_to([RP, B_LOC, nmax])
                D = npool.tile([RP, B_LOC, nmax], f32, tag="D")
                nc.vector.tensor_tensor(D[:], ci3, rc3, op=OP.subtract)
                aD = npool.tile([RP, B_LOC, nmax], f32, tag="aD")
                nc.vector.scalar_tensor_tensor(
                    aD[:], D[:], -1.0, D[:], op0=OP.mult, op1=OP.max
                )  # |j-c|
                tv = npool.tile([RP, B_LOC, nmax], f32, tag="tv")
                nc.vector.tensor_scalar(
                    tv[:], aD[:], k_ap, 0.0, op0=OP.subtract, op1=OP.min
                )  # min(|j-c|-k, 0)
                sp = npool.tile([RP, B_LOC, nmax], f32, tag="sp")
                nc.vector.tensor_tensor(
                    sp[:], tv[:], u[:].unsqueeze(2).broadcast_to([RP, B_LOC, nmax]),
                    op=OP.mult,
                )
                pen = npool.tile([RP, B_LOC, nmax], f32, tag="pen")
                nc.vector.tensor_scalar(
                    pen[:], sp[:], tau_ap, -PEN, op0=OP.is_gt, op1=OP.mult
                )
                nc.vector.tensor_tensor(S3, S3, pen[:], op=OP.add)

            # ---- stage outputs (host gathers pi from the raw grid dump) ----
            nc.sync.dma_start(ps_out[:], stage_sc[:, 0:n_prop])
            nc.scalar.dma_start(pir_out[:], stage85[:])

    nc.compile()
    return nc, hc


def _prepare(cfg, input_tensor, consts):
    in_maps = []
    for core in range(N_CORES):
        shard = np.ascontiguousarray(
            input_tensor[core * B_LOC : (core + 1) * B_LOC].reshape(B_LOC, C, HW)
        ).astype(np.float32, copy=False)
        m = {
            "x": shard,
            "acat": consts["acat"],
            "ident": consts["ident"],
            "cgrid": consts["cgrid"],
            "inde": consts["inde"],
            "indr": consts["indr"],
            "crow": consts["crow"],
        }
        in_maps.append(m)
    return in_maps


def kernel(
    num_proposals,
    input_tensor,
    window_nums_sum,
    N_list,
    iou_thresholds,
    coordinates_cat,
):
    from concourse.bass_utils import run_bass_kernel_spmd

    input_tensor = np.asarray(input_tensor)
    wns = tuple(int(v) for v in np.asarray(window_nums_sum))
    nl = tuple(int(v) for v in np.asarray(N_list))
    ths = tuple(float(v) for v in np.asarray(iou_thresholds))
    cfg = (wns, nl, ths)

    # sanity: boxes must be the regular grid this kernel specializes for
    hc_probe = _host_consts(cfg)
    exp = _expected_grid_coords(wns, hc_probe["ns"], hc_probe["ks"])
    cc = np.asarray(coordinates_cat, dtype=np.float32)
    assert cc.shape == exp.shape and np.allclose(cc, exp), (
        "coordinates_cat is not the expected sliding-window grid"
    )

    nc, _hc = _build(cfg)
    in_maps = _prepare(cfg, input_tensor, _hc)
    res = run_bass_kernel_spmd(nc, in_maps, core_ids=list(range(N_CORES))).results

    n_prop = sum(nl)
    n_iter = max(nl)
    wtot = wns[-1]
    goff = _hc["goff"]
    slot_base = [sum(nl[:g]) for g in range(len(nl))]
    pi = np.zeros((B, n_prop), np.int32)
    for core in range(N_CORES):
        pir = res[core]["pir"]  # [RP, 2*n_iter], value at [goff[g], 2t+b]
        for g in range(len(nl)):
            for t in range(nl[g]):
                for bb in range(B_LOC):
                    pi[core * B_LOC + bb, slot_base[g] + t] = int(
                        pir[goff[g], B_LOC * t + bb]
                    )
    ps = np.concatenate([res[i]["ps"] for i in range(N_CORES)], 0).astype(np.float32)
    ws = np.concatenate([res[i]["ws"] for i in range(N_CORES)], 0).astype(np.float32)
    assert pi.shape == (B, n_prop) and ws.shape == (B, wtot)
    return pi, ps, ws


# revision 23
# speedup vs baseline: 1.0855x; 1.0674x over previous
"""Trainium2 Bass kernel for AdaptivePooling + NMS (nn_AdaptivePoolingAndNMS).

Reference semantics:
  x = input_tensor.sum(axis=1)                               # [B, 28, 28]
  scores_k = avgpool2d(x, k, stride 1, VALID).reshape(B, -1) # k in {4, 6, 8}
  all_scores = concat(scores_k)                              # [B, 1595]
  per group g: greedy NMS picking N_list[g] windows (IoU thresh)
  returns (proposal_indices [B,6] i32, proposal_scores [B,6] f32,
           window_scores [B,1595] f32)

Sharding: pure data parallel, batch dim over 8 cores (2 per core).

Device algorithm per core (B_loc = 2): see inline comments. Grid rows of the
three pooled maps are stacked on partitions at 32-aligned offsets (0/32/64)
because engine access patterns may only start at partitions 0/32/64/96.
"""

import functools

import numpy as np

H = W = 28
B, C = 16, 2048
N_CORES = 8
B_LOC = B // N_CORES
HW = H * W
CHUNKS = C // 128          # 16 K-chunks of 128 channels
CPT = 4                    # channels per partition per DMA tile (contiguous)
N_DMAS = C // (128 * CPT)  # 4 DMAs per batch, [128, 4, 784] = 1.57 MB each
RPAD = 96                  # padded row count for transposed NMS tiles
BIGE = 65536.0             # encoding offset (exact in f32 for small ints)
PEN = 1.0e9                # suppression penalty / padding


def _expected_grid_coords(wns, ns, ks, stride=16):
    coords = []
    for g in range(len(ns)):
        n, k = ns[g], ks[g]
        ii, jj = np.meshgrid(np.arange(n), np.arange(n), indexing="ij")
        boxes = (
            np.stack([jj, ii, jj + k, ii + k], -1).reshape(-1, 4).astype(np.float32)
            * stride
        )
        coords.append(boxes)
    return np.concatenate(coords, 0)


def _host_consts(cfg):
    """Build the small constant arrays uploaded to each core."""
    wns, nl, ths = cfg
    n_groups = len(nl)
    ns, ks = [], []
    for g in range(n_groups):
        size = wns[g + 1] - wns[g]
        n = int(round(np.sqrt(size)))
        assert n * n == size, f"group {g} size {size} not square"
        assert n <= 32
        ns.append(n)
        ks.append(H - n + 1)
    goff = [32 * g for g in range(n_groups)]   # partition-aligned group rows
    RP = goff[-1] + ns[-1]                     # total padded rows (85)
    assert RP <= 128

    # vertical pooling matrix A_cat [28, RP] (zero cols on gap rows)
    acat = np.zeros((H, RP), np.float32)
    for g in range(n_groups):
        n, k = ns[g], ks[g]
        for i in range(n):
            acat[i : i + k, goff[g] + i] = 1.0

    ident = np.eye(128, dtype=np.float32)

    nmax = max(ns)
    # cgrid [RP, 4*nmax + 6]: packed per-row constants
    #   0:nmax        CIOTA_MB = j - BIGE  (batch 0)
    #   nmax:2nmax    CIOTA_MB (batch 1)
    #   2n:3n         CIOTA    = j         (batch 0)
    #   3n:4n         CIOTA    (batch 1)
    #   4n+0          ROWBASE64 = 64*row_in_group
    #   4n+1          IOTAROW   = row_in_group
    #   4n+2          K         = k_g
    #   4n+3          TAU       = 2*t*k^2/(1+t)
    #   4n+4          N_AP      = n_g
    #   4n+5          WNS_AP    = wns_g
    cg = np.zeros((RP, 4 * nmax + 6), np.float32)
    for r in range(RP):
        cg[r, 0:nmax] = np.arange(nmax) - BIGE
        cg[r, nmax : 2 * nmax] = np.arange(nmax) - BIGE
        cg[r, 2 * nmax : 3 * nmax] = np.arange(nmax)
        cg[r, 3 * nmax : 4 * nmax] = np.arange(nmax)
    for g in range(n_groups):
        n, k, t = ns[g], ks[g], ths[g]
        tau = 2.0 * t * k * k / (1.0 + t)
        for i in range(n):
            r = goff[g] + i
            cg[r, 4 * nmax + 0] = 64.0 * i
            cg[r, 4 * nmax + 1] = float(i)
            cg[r, 4 * nmax + 2] = float(k)
            cg[r, 4 * nmax + 3] = tau
            cg[r, 4 * nmax + 4] = float(n)
            cg[r, 4 * nmax + 5] = float(wns[g])

    # inde [n_groups, RP]: broadcast selector (group -> its grid rows)
    inde = np.zeros((n_groups, RP), np.float32)
    for g in range(n_groups):
        inde[g, goff[g] : goff[g] + ns[g]] = 1.0

    return {
        "ns": ns,
        "ks": ks,
        "goff": goff,
        "RP": RP,
        "nmax": nmax,
        "acat": acat,
        "ident": ident,
        "cgrid": cg,
        "inde": inde,
    }


@functools.lru_cache(maxsize=4)
def _build(cfg):
    """Build + compile the per-core Bass program. cfg = (wns, nl, ths)."""
    import concourse.bacc as bacc
    import concourse.mybir as mybir
    from concourse.tile import TileContext

    wns, nl, ths = cfg
    hc = _host_consts(cfg)
    ns, ks, goff = hc["ns"], hc["ks"], hc["goff"]
    RP, nmax = hc["RP"], hc["nmax"]
    n_groups = len(nl)
    n_iter = max(nl)
    n_prop = sum(nl)
    slot_base = [sum(nl[:g]) for g in range(n_groups)]
    wtot = wns[-1]

    f32 = mybir.dt.float32
    i32 = mybir.dt.int32

    nc = bacc.Bacc(None, target_bir_lowering=False, debug=False)

    x_in = nc.declare_dram_parameter("x", [B_LOC, C, HW], f32, isOutput=False)
    acat_in = nc.declare_dram_parameter("acat", [H, RP], f32, isOutput=False)
    ident_in = nc.declare_dram_parameter("ident", [128, 128], f32, isOutput=False)
    cgrid_in = nc.declare_dram_parameter(
        "cgrid", [RP, 4 * nmax + 6], f32, isOutput=False
    )
    inde_in = nc.declare_dram_parameter(
        "inde", [n_groups, RP], f32, isOutput=False
    )

    pir_out = nc.declare_dram_parameter(
        "pir", [RP, 2 * n_iter], f32, isOutput=True
    )
    ps_out = nc.declare_dram_parameter("ps", [B_LOC, n_prop], f32, isOutput=True)
    ws_out = nc.declare_dram_parameter("ws", [B_LOC, wtot], f32, isOutput=True)

    AX = mybir.AxisListType.X
    OP = mybir.AluOpType
    ACTF = mybir.ActivationFunctionType

    with TileContext(nc) as tc:
        with (
            tc.tile_pool(name="const", bufs=1) as cpool,
            tc.tile_pool(name="xin", bufs=8) as xpool,
            tc.tile_pool(name="pairp", bufs=2) as ppool,
            tc.tile_pool(name="work", bufs=1) as wpool,
            tc.tile_pool(name="nms", bufs=2) as npool,
            tc.tile_pool(name="psacc", bufs=2, space="PSUM") as pacc,
            tc.tile_pool(name="psvp", bufs=1, space="PSUM") as pvp,
            tc.tile_pool(name="psnms", bufs=1, space="PSUM") as pnms,
        ):
            # ---- constants ----
            ones = cpool.tile([128, 1], f32)
            nc.vector.memset(ones[:], 1.0)
            acat = cpool.tile([H, RP], f32)
            nc.gpsimd.dma_start(acat[:], acat_in[:])
            ident = cpool.tile([128, 128], f32)
            nc.gpsimd.dma_start(ident[:], ident_in[:])
            cgrid = cpool.tile([RP, 4 * nmax + 6], f32)
            nc.gpsimd.dma_start(cgrid[:], cgrid_in[:])
            inde = cpool.tile([n_groups, RP], f32)
            nc.gpsimd.dma_start(inde[:], inde_in[:])

            ciota_mb = cgrid[:, 0 : 2 * nmax]
            ciota = cgrid[:, 2 * nmax : 4 * nmax]
            rowbase64 = cgrid[:, 4 * nmax : 4 * nmax + 1]
            iotarow = cgrid[:, 4 * nmax + 1 : 4 * nmax + 2]
            k_ap = cgrid[:, 4 * nmax + 2 : 4 * nmax + 3]
            tau_ap = cgrid[:, 4 * nmax + 3 : 4 * nmax + 4]
            n_ap = cgrid[:, 4 * nmax + 4 : 4 * nmax + 5]
            wns_ap = cgrid[:, 4 * nmax + 5 : 4 * nmax + 6]

            # ---- S: score grid [RP, 2, nmax] (free = (batch, col)) ----
            S = wpool.tile([RP, B_LOC, nmax], f32)
            nc.vector.memset(S[:], -PEN)

            # ---- channel sum + pooling, per batch ----
            for b in range(B_LOC):
                # partition p of DMA tile j holds channels 512j+4p..512j+4p+3
                # (contiguous per partition -> 1 DMA descriptor per partition)
                xb = x_in[b].rearrange("(j p ci) m -> j p ci m", p=128, ci=CPT)
                tiles = []
                for d in range(N_DMAS):
                    xt = xpool.tile([128, CPT, HW], f32, tag="xt")
                    # alternate the two HWDGE rings (sync + scalar queues)
                    eng = nc.sync if d % 2 == 0 else nc.scalar
                    eng.dma_start(xt[:], xb[d])
                    tiles.append(xt)

                # DVE add tree per tile: 4 chans -> 2 pairs -> 1 quad;
                # quads chain-accumulate to one [128, HW] tile; PE does only
                # the final 128->1 reduce (4 fp32 passes)
                quads = []
                for d in range(N_DMAS):
                    pa = ppool.tile([128, HW], f32, tag="pa")
                    nc.vector.tensor_add(pa[:], tiles[d][:, 0, :], tiles[d][:, 1, :])
                    pb = ppool.tile([128, HW], f32, tag="pb")
                    nc.vector.tensor_add(pb[:], tiles[d][:, 2, :], tiles[d][:, 3, :])
                    quad = ppool.tile([128, HW], f32, tag="quad")
                    nc.vector.tensor_add(quad[:], pa[:], pb[:])
                    quads.append(quad)
                oc0 = ppool.tile([128, HW], f32, tag="oc0")
                nc.vector.tensor_add(oc0[:], quads[0][:], quads[1][:])
                fin = ppool.tile([128, HW], f32, tag="fin")
                nc.vector.tensor_add(fin[:], quads[2][:], quads[3][:])
                nc.vector.tensor_add(fin[:], fin[:], oc0[:])
                acc0 = pacc.tile([1, HW // 2], f32, tag="acc")
                acc1 = pacc.tile([1, HW // 2], f32, tag="acc")
                nc.tensor.matmul(
                    acc0[:], ones[:], fin[:, 0 : HW // 2], start=True, stop=True
                )
                nc.tensor.matmul(
                    acc1[:], ones[:], fin[:, HW // 2 : HW], start=True, stop=True
                )
                # reshape [1, 784] -> [28, 28] (PSUM -> SBUF flat -> DMA reshape)
                xflat = wpool.tile([1, HW], f32, tag="xflat")
                nc.vector.tensor_copy(xflat[:, 0 : HW // 2], acc0[:])
                nc.vector.tensor_copy(xflat[:, HW // 2 : HW], acc1[:])
                xs = wpool.tile([H, W], f32, tag="xs")
                nc.sync.dma_start(xs[:], xflat[:])

                # vertical pooling (gap rows get zero columns in acat)
                vp = pvp.tile([RP, W], f32, tag="vp")
                nc.tensor.matmul(vp[:], acat[:], xs[:], start=True, stop=True)

                # horizontal pooling (shifted adds) into S[:, b, :]
                vps = wpool.tile([RP, W], f32, tag="vps")
                nc.vector.tensor_copy(vps[:], vp[:])
                t = wpool.tile([RP, W - 1], f32, tag="hp_t")
                nc.vector.tensor_add(t[:], vps[:, 0 : W - 1], vps[:, 1:W])
                s4 = wpool.tile([RP, W - 3], f32, tag="hp_s4")
                nc.vector.tensor_add(s4[:], t[:, 0 : W - 3], t[:, 2 : W - 1])
                for g in range(n_groups):
                    n, k = ns[g], ks[g]
                    sl = slice(goff[g], goff[g] + n)
                    if k == 4:
                        src = s4[sl, 0:n]
                    else:
                        tmp = wpool.tile([RP, nmax], f32, tag=f"hp_k{k}")
                        if k == 6:
                            nc.vector.tensor_add(
                                tmp[sl, 0:n], s4[sl, 0:n], t[sl, 4 : 4 + n]
                            )
                        elif k == 8:
                            nc.vector.tensor_add(
                                tmp[sl, 0:n], s4[sl, 0:n], s4[sl, 4 : 4 + n]
                            )
                        else:
                            raise NotImplementedError(f"k={k}")
                        src = tmp[sl, 0:n]
                    nc.scalar.mul(S[sl, b, 0:n], src, 1.0 / (k * k))

            # ---- window_scores out ----
            for g in range(n_groups):
                n = ns[g]
                dst = ws_out[:, wns[g] : wns[g] + n * n].rearrange(
                    "b (i j) -> i b j", i=n
                )
                nc.sync.dma_start(dst, S[goff[g] : goff[g] + n, :, 0:n])

            # ---- NMS ----
            # working copy so the ws DMAs don't gate suppression writes
            Sw = wpool.tile([RP, B_LOC, nmax], f32, tag="Sw")
            nc.vector.tensor_copy(
                Sw[:].rearrange("r b j -> r (b j)"),
                S[:].rearrange("r b j -> r (b j)"),
            )
            S3 = Sw[:]  # [RP, 2, nmax]

            stage_sc = npool.tile([B_LOC, n_prop], f32, tag="stage_sc")
            stage85 = npool.tile([RP, 2 * n_iter], f32, tag="stage85")
            # persistent padded argmax tiles ([RPAD, 2]; pad rows preset once)
            rm = wpool.tile([RPAD, B_LOC], f32, tag="rm")
            nc.vector.memset(rm[:], -PEN)
            en = wpool.tile([RPAD, B_LOC], f32, tag="en")
            nc.vector.memset(en[:], 0.0)

            for it in range(n_iter):
                # per-row max + argmax-encoding enc' = 64r + c - BIGE
                nc.vector.tensor_reduce(rm[0:RP, :], S3, axis=AX, op=OP.max)
                eq = npool.tile([RP, B_LOC, nmax], f32, tag="eq")
                nc.vector.tensor_tensor(
                    eq[:],
                    S3,
                    rm[0:RP, :].unsqueeze(2).broadcast_to([RP, B_LOC, nmax]),
                    op=OP.is_equal,
                )
                tm = npool.tile([RP, B_LOC, nmax], f32, tag="tm")
                nc.vector.tensor_tensor(
                    tm[:], eq[:], ciota_mb.rearrange("r (b j) -> r b j", b=B_LOC),
                    op=OP.mult,
                )
                nc.vector.tensor_reduce(en[0:RP, :], tm[:], axis=AX, op=OP.min)
                nc.vector.tensor_scalar(
                    en[0:RP, :], en[0:RP, :], rowbase64, None, op0=OP.add
                )
                # transpose to [2, RPAD]; per-group reductions read PSUM direct
                Q1 = pnms.tile([B_LOC, RPAD], f32, tag="Q1")
                nc.tensor.transpose(Q1[:], rm[:], ident[0:RPAD, 0:RPAD])
                Q2 = pnms.tile([B_LOC, RPAD], f32, tag="Q2")
                nc.tensor.transpose(Q2[:], en[:], ident[0:RPAD, 0:RPAD])
                q1g = Q1[:].rearrange("b (g n) -> b g n", g=n_groups)
                q2g = Q2[:].rearrange("b (g n) -> b g n", g=n_groups)

                m2 = npool.tile([B_LOC, n_groups], f32, tag="m2")
                nc.vector.tensor_reduce(m2[:], q1g, axis=AX, op=OP.max)
                eq2 = npool.tile([B_LOC, n_groups, 32], f32, tag="eq2")
                nc.vector.tensor_tensor(
                    eq2[:],
                    q1g,
                    m2[:].unsqueeze(2).broadcast_to([B_LOC, n_groups, 32]),
                    op=OP.is_equal,
                )
                # F [2, G]: per-group min of enc' (= enc - BIGE, < 0)
                F = npool.tile([B_LOC, n_groups], f32, tag="F")
                t3 = npool.tile([B_LOC, n_groups, 32], f32, tag="t3")
                nc.vector.tensor_tensor(t3[:], eq2[:], q2g, op=OP.mult)
                nc.vector.tensor_reduce(F[:], t3[:], axis=AX, op=OP.min)
                # stage proposal scores for still-active groups
                for g in range(n_groups):
                    if it < nl[g]:
                        nc.scalar.copy(
                            stage_sc[:, slot_base[g] + it : slot_base[g] + it + 1],
                            m2[:, g : g + 1],
                        )

                # transpose F -> [G, 2]; +BIGE; broadcast enc to grid rows;
                # then r = enc >> 6, c = enc & 63 via int32 ops on DVE
                Ft = pnms.tile([n_groups, 2], f32, tag="Ft")
                nc.tensor.transpose(Ft[:], F[:], ident[0:2, 0:2])
                fc = npool.tile([n_groups, 2], f32, tag="fc")
                nc.vector.tensor_scalar(fc[:], Ft[:], BIGE, None, op0=OP.add)
                Renc = pnms.tile([RP, B_LOC], f32, tag="Renc")
                nc.tensor.matmul(Renc[:], inde[:], fc[:], start=True, stop=True)
                enci = npool.tile([RP, B_LOC], i32, tag="enci")
                nc.vector.tensor_copy(enci[:], Renc[:])
                ri = npool.tile([RP, B_LOC], i32, tag="ri")
                nc.vector.tensor_scalar(
                    ri[:], enci[:], 6, None, op0=OP.arith_shift_right
                )
                rrs = npool.tile([RP, B_LOC], f32, tag="rrs")
                nc.vector.tensor_copy(rrs[:], ri[:])
                ci = npool.tile([RP, B_LOC], i32, tag="ci")
                nc.vector.tensor_scalar(
                    ci[:], enci[:], 63, None, op0=OP.bitwise_and
                )
                rcs = npool.tile([RP, B_LOC], f32, tag="rcs")
                nc.vector.tensor_copy(rcs[:], ci[:])
                # output indices: oi = r*n_g + wns_g + c (valid on group rows)
                oi = npool.tile([RP, B_LOC], f32, tag="oi")
                nc.vector.tensor_scalar(
                    oi[:], rrs[:], n_ap, wns_ap, op0=OP.mult, op1=OP.add
                )
                nc.vector.tensor_tensor(
                    stage85[:, B_LOC * it : B_LOC * (it + 1)],
                    oi[:],
                    rcs[:],
                    op=OP.add,
                )

                if it == n_iter - 1:
                    break

                # suppression: (k-|i-r|)+ * (k-|j-c|)+ > tau  -> S -= PEN
                # (computed sign-flipped: min(|d|-k, 0) * min(|e|-k, 0))
                d_ = npool.tile([RP, B_LOC], f32, tag="d_")
                nc.vector.tensor_tensor(
                    d_[:], iotarow.broadcast_to([RP, B_LOC]), rrs[:], op=OP.subtract
                )
                ad = npool.tile([RP, B_LOC], f32, tag="ad")
                nc.vector.scalar_tensor_tensor(
                    ad[:], d_[:], -1.0, d_[:], op0=OP.mult, op1=OP.max
                )  # |i-r|
                u = npool.tile([RP, B_LOC], f32, tag="u")
                nc.vector.tensor_scalar(
                    u[:], ad[:], k_ap, 0.0, op0=OP.subtract, op1=OP.min
                )  # min(|i-r|-k, 0)
                ci3 = ciota.rearrange("r (b j) -> r b j", b=B_LOC)
                rc3 = rcs[:].unsqueeze(2).broadcast_to([RP, B_LOC, nmax])
                D = npool.tile([RP, B_LOC, nmax], f32, tag="D")
                nc.vector.tensor_tensor(D[:], ci3, rc3, op=OP.subtract)
                aD = npool.tile([RP, B_LOC, nmax], f32, tag="aD")
                nc.vector.scalar_tensor_tensor(
                    aD[:], D[:], -1.0, D[:], op0=OP.mult, op1=OP.max
                )  # |j-c|
                tv = npool.tile([RP, B_LOC, nmax], f32, tag="tv")
                nc.vector.tensor_scalar(
                    tv[:], aD[:], k_ap, 0.0, op0=OP.subtract, op1=OP.min
                )  # min(|j-c|-k, 0)
                sp = npool.tile([RP, B_LOC, nmax], f32, tag="sp")
                nc.vector.tensor_tensor(
                    sp[:], tv[:], u[:].unsqueeze(2).broadcast_to([RP, B_LOC, nmax]),
                    op=OP.mult,
                )
                pen = npool.tile([RP, B_LOC, nmax], f32, tag="pen")
                nc.vector.tensor_scalar(
                    pen[:], sp[:], tau_ap, -PEN, op0=OP.is_gt, op1=OP.mult
                )
                nc.vector.tensor_tensor(S3, S3, pen[:], op=OP.add)

            # ---- stage outputs (host gathers pi from the raw grid dump) ----
            nc.sync.dma_start(ps_out[:], stage_sc[:, 0:n_prop])
            nc.scalar.dma_start(pir_out[:], stage85[:])

    nc.compile()
    return nc, hc


def _prepare(cfg, input_tensor, consts):
    in_maps = []
    for core in range(N_CORES):
        shard = np.ascontiguousarray(
            input_tensor[core * B_LOC : (core + 1) * B_LOC].reshape(B_LOC, C, HW)
        ).astype(np.float32, copy=False)
        m = {
            "x": shard,
            "acat": consts["acat"],
            "ident": consts["ident"],
            "cgrid": consts["cgrid"],
            "inde": consts["inde"],
        }
        in_maps.append(m)
    return in_maps


def kernel(
    num_proposals,
    input_tensor,
    window_nums_sum,
    N_list,
    iou_thresholds,
    coordinates_cat,
):
    from concourse.bass_utils import run_bass_kernel_spmd

    input_tensor = np.asarray(input_tensor)
    wns = tuple(int(v) for v in np.asarray(window_nums_sum))
    nl = tuple(int(v) for v in np.asarray(N_list))
    ths = tuple(float(v) for v in np.asarray(iou_thresholds))
    cfg = (wns, nl, ths)

    # sanity: boxes must be the regular grid this kernel specializes for
    hc_probe = _host_consts(cfg)
    exp = _expected_grid_coords(wns, hc_probe["ns"], hc_probe["ks"])
    cc = np.asarray(coordinates_cat, dtype=np.float32)
    assert cc.shape == exp.shape and np.allclose(cc, exp), (
        "coordinates_cat is not the expected sliding-window grid"
    )

    nc, _hc = _build(cfg)
    in_maps = _prepare(cfg, input_tensor, _hc)
    res = run_bass_kernel_spmd(nc, in_maps, core_ids=list(range(N_CORES))).results

    n_prop = sum(nl)
    n_iter = max(nl)
    wtot = wns[-1]
    goff = _hc["goff"]
    slot_base = [sum(nl[:g]) for g in range(len(nl))]
    pi = np.zeros((B, n_prop), np.int32)
    for core in range(N_CORES):
        pir = res[core]["pir"]  # [RP, 2*n_iter], value at [goff[g], 2t+b]
        for g in range(len(nl)):
            for t in range(nl[g]):
                for bb in range(B_LOC):
                    pi[core * B_LOC + bb, slot_base[g] + t] = int(
                        pir[goff[g], B_LOC * t + bb]
                    )
    ps = np.concatenate([res[i]["ps"] for i in range(N_CORES)], 0).astype(np.float32)
    ws = np.concatenate([res[i]["ws"] for i in range(N_CORES)], 0).astype(np.float32)
    assert pi.shape == (B, n_prop) and ws.shape == (B, wtot)
    return pi, ps, ws
